# revision 16
# baseline (speedup 1.0000x reference)
import numpy as np
import ml_dtypes

import concourse.bass as bass
import concourse.bacc as bacc
import concourse.mybir as mybir
import concourse.tile as tile
from concourse import bass_utils

BF16 = mybir.dt.bfloat16
F32 = mybir.dt.float32
I32 = mybir.dt.int32
AF = mybir.ActivationFunctionType
ALU = mybir.AluOpType

S = 2048
HID = 768
D = 64
NH = 6
NPAIR = 3
NRB = 4
RB = 512
HB = 256
CT = 6
LN_EPS = 1e-8
N_CORES = 8
RSQRT_MAGIC = 0x5F3759DF


def _rope_tables():
    inv_freq = 1.0 / (10000.0 ** (np.arange(0, D, 2, dtype=np.float64) / D))
    t = np.arange(S, dtype=np.float64)
    freqs = np.outer(t, inv_freq)
    emb = np.concatenate([freqs, freqs], axis=-1)
    return np.cos(emb).astype(np.float32), np.sin(emb).astype(np.float32)


def build_nc(ndev, pairs):
    nc = bacc.Bacc("TRN2", target_bir_lowering=False, debug=False,
                   num_devices=ndev)

    def din(name, shape, dt):
        return nc.dram_tensor(name, shape, dt, kind="ExternalInput").ap()

    xT = din("xT", [HID, S], BF16)
    w_qkv = din("w_qkv", [HID, 3 * NH * D], BF16)
    w_u = din("w_u", [HID, NPAIR * 128], BF16)
    w_out = din("w_out", [NPAIR * 128, HID], BF16)
    cosr = din("cosr", [S, NH * D], BF16)
    sinr = din("sinr", [S, NH * D], BF16)
    maskb = din("maskb", [128, 128], BF16)
    ones_k = din("ones_k", [128, 1], BF16)
    residT = din("residT", [HID, S], BF16)
    out = nc.dram_tensor("out", [NRB, CT, 128, HB], BF16,
                         kind="ExternalOutput").ap()

    xT_r = xT.rearrange("(k p) s -> k p s", p=128)
    wqkv_r = w_qkv.rearrange("(k p) c -> k p c", p=128)
    wu_r = w_u.rearrange("(k p) c -> k p c", p=128)
    wout_r = w_out.rearrange("(k p) c -> k p c", p=128)
    residT_r = residT.rearrange("(c p) s -> p c s", p=128)

    with tile.TileContext(nc) as tc:
        _emit(nc, tc, pairs, xT_r, wqkv_r, wu_r, wout_r, cosr, sinr,
              maskb, ones_k, residT_r, out)
    nc.compile()
    return nc


def _emit(nc, tc, pairs, xT_r, wqkv_r, wu_r, wout_r, cosr, sinr,
          maskb, ones_k, residT_r, out):
    from contextlib import ExitStack
    es = ExitStack()
    with es:
        res = es.enter_context(tc.tile_pool(name="resident", bufs=1))
        xT_sb = [res.tile([128, S], BF16, tag=f"xT{k}", name=f"xT{k}")
                 for k in range(6)]
        wqkv_sb = [res.tile([128, 3 * NH * D], BF16, tag=f"wqkv{k}",
                            name=f"wqkv{k}") for k in range(6)]
        wu_sb = [res.tile([128, NPAIR * 128], BF16, tag=f"wu{k}",
                          name=f"wu{k}") for k in range(6)]
        wout_sb = [res.tile([128, HID], BF16, tag=f"wout{k}",
                            name=f"wout{k}") for k in range(NPAIR)]
        maskb_sb = res.tile([128, 128], BF16, tag="maskb")
        ones_k_sb = res.tile([128, 1], BF16, tag="onesk")
        qt_sb = [res.tile([128, NPAIR, RB], BF16, tag=f"qt{i}", name=f"qt{i}")
                 for i in range(NRB)]
        kt_sb = [res.tile([128, NPAIR, RB], BF16, tag=f"kt{i}", name=f"kt{i}")
                 for i in range(NRB)]
        v_sb = [res.tile([128, 4, NH, D], BF16, tag=f"v{i}", name=f"v{i}")
                for i in range(NRB)]
        ao_sb = [res.tile([128, NPAIR, RB], BF16, tag=f"ao{i}", name=f"ao{i}")
                 for i in range(NRB)]
        ut_sb = [[res.tile([128, 2, RB], BF16, tag=f"ut{p}_{sp}",
                           name=f"ut{p}_{sp}")
                  for sp in range(2)] for p in range(NPAIR)]

        for k in range(6):
            nc.sync.dma_start(out=xT_sb[k][:], in_=xT_r[k])
            nc.sync.dma_start(out=wqkv_sb[k][:], in_=wqkv_r[k])
        nc.sync.dma_start(out=maskb_sb[:], in_=maskb[:])
        nc.sync.dma_start(out=ones_k_sb[:], in_=ones_k[:])
        for k in range(6):
            nc.sync.dma_start(out=wu_sb[k][:], in_=wu_r[k])
        for p in range(NPAIR):
            nc.sync.dma_start(out=wout_sb[p][:], in_=wout_r[p])

        dram = es.enter_context(tc.tile_pool(name="ccdram", bufs=4,
                                             space="DRAM"))
        scp = es.enter_context(tc.tile_pool(name="p2sc", bufs=2,
                                            space="PSUM"))
        avp = es.enter_context(tc.tile_pool(name="p2av", bufs=1,
                                            space="PSUM"))
        atp = es.enter_context(tc.tile_pool(name="p2sb", bufs=16))
        sb = es.enter_context(tc.tile_pool(name="p1sb", bufs=2))
        with tc.tile_pool(name="p1psum", bufs=1, space="PSUM") as pp:
            for rt in range(16):
                pqk = pp.tile([128, 3, 512], F32, tag="pqk")
                for k in range(6):
                    lhs = xT_sb[k][:, rt * 128:(rt + 1) * 128]
                    st, sp = (k == 0), (k == 5)
                    nc.tensor.matmul(pqk[:, 0, 0:384], lhs,
                                     wqkv_sb[k][:, 0:384], start=st, stop=sp)
                    nc.tensor.matmul(pqk[:, 1, 0:384], lhs,
                                     wqkv_sb[k][:, 384:768], start=st,
                                     stop=sp)
                    nc.tensor.matmul(pqk[:, 2, 0:384], lhs,
                                     wqkv_sb[k][:, 768:1152], start=st,
                                     stop=sp)
                rb, rt4 = rt // 4, rt % 4
                r0, r1 = rt * 128, (rt + 1) * 128
                c0, c1 = rt4 * 128, (rt4 + 1) * 128
                q_t = sb.tile([128, NH, D], BF16, tag="qrow")
                k_t = sb.tile([128, NH, D], BF16, tag="krow")
                nc.scalar.copy(q_t[:], pqk[:, 0, 0:384])
                nc.scalar.copy(k_t[:], pqk[:, 1, 0:384])
                nc.vector.tensor_copy(v_sb[rb][:, rt4, :, :],
                                      pqk[:, 2, 0:384])
                cos_t = sb.tile([128, NH, D], BF16, tag="cos")
                sin_t = sb.tile([128, NH, D], BF16, tag="sin")
                nc.sync.dma_start(out=cos_t[:], in_=cosr[r0:r1, :])
                nc.sync.dma_start(out=sin_t[:], in_=sinr[r0:r1, :])
                for name, src in (("q", q_t), ("k", k_t)):
                    t1 = sb.tile([128, NH, D], BF16, tag="rope_t1")
                    t2 = sb.tile([128, NH, D], BF16, tag="rope_t2")
                    rr = sb.tile([128, NH, D], BF16, tag="rope_r")
                    nc.vector.tensor_mul(t1[:, :, 0:32], src[:, :, 32:64],
                                         sin_t[:, :, 0:32])
                    nc.vector.tensor_mul(t1[:, :, 32:64], src[:, :, 0:32],
                                         sin_t[:, :, 32:64])
                    nc.vector.tensor_mul(t2[:], src[:], cos_t[:])
                    nc.vector.tensor_add(rr[:], t1[:], t2[:])
                    dst = qt_sb if name == "q" else kt_sb
                    for p in range(NPAIR):
                        nc.sync.dma_start_transpose(
                            out=dst[rb][:, p, c0:c1],
                            in_=rr[:, 2 * p:2 * p + 2, :])

            for p in range(NPAIR):
                for sp in range(2):
                    pu = pp.tile([128, 2, 512], F32, tag="pqk", name="pu")
                    for k in range(6):
                        lhsw = wu_sb[k][:, p * 128:(p + 1) * 128]
                        st, spp = (k == 0), (k == 5)
                        nc.tensor.matmul(
                            pu[:, 0, :], lhsw,
                            xT_sb[k][:, 2 * sp * RB:2 * sp * RB + RB],
                            start=st, stop=spp)
                        nc.tensor.matmul(
                            pu[:, 1, :], lhsw,
                            xT_sb[k][:, (2 * sp + 1) * RB:
                                       (2 * sp + 2) * RB],
                            start=st, stop=spp)
                    usig = sb.tile([128, 2, RB], BF16, tag="usig")
                    nc.scalar.activation(usig[:], pu[:], AF.Sigmoid)
                    nc.vector.tensor_mul(ut_sb[p][sp][:], usig[:], pu[:])

        stp = es.enter_context(tc.tile_pool(name="p3st", bufs=1,
                                            space="PSUM"))
        opo = es.enter_context(tc.tile_pool(name="p3o", bufs=1,
                                            space="PSUM"))
        sb3 = es.enter_context(tc.tile_pool(name="p3sb", bufs=2))
        ssb = es.enter_context(tc.tile_pool(name="p3small", bufs=1))
        for qb in range(NRB):
            nkc = 4 * qb + 4
            for p in range(NPAIR):
                av = avp.tile([128, RB], F32, tag="av")
                ats = {}
                for kc in range(nkc):
                    t = kc - 4 * qb
                    q0 = max(t, 0) * 128
                    sc = scp.tile([128, 2, RB], F32, tag="sc")
                    at = atp.tile([128, 2, RB], BF16, tag="at")
                    kslc = kt_sb[kc // 4]
                    for h01 in range(2):
                        b0 = 64 * h01
                        nc.tensor.matmul(
                            sc[:, h01, :],
                            kslc[b0:b0 + 64, p,
                                 (kc % 4) * 128:(kc % 4 + 1) * 128],
                            qt_sb[qb][b0:b0 + 64, p, :],
                            start=True, stop=True)
                    nc.scalar.activation(at[:, :, q0:RB], sc[:, :, q0:RB],
                                         AF.Sigmoid, scale=0.125)
                    if t >= 0:
                        if q0 > 0:
                            nc.vector.memset(at[:, :, 0:q0], 0.0)
                        for h01 in range(2):
                            nc.vector.tensor_mul(
                                at[:, h01, q0:q0 + 128],
                                at[:, h01, q0:q0 + 128],
                                maskb_sb[:])
                    ats[kc] = at
                for kc in range(nkc):
                    at = ats[kc]
                    for h01 in range(2):
                        b0 = 64 * h01
                        nc.tensor.matmul(
                            av[b0:b0 + 64, :],
                            v_sb[kc // 4][:, kc % 4, 2 * p + h01, :],
                            at[:, h01, :],
                            start=(kc == 0), stop=(kc == nkc - 1),
                            skip_group_check=True)
                nc.vector.tensor_copy(ao_sb[qb][:, p, :], av[:])

            aof = ao_sb[qb]
            ssum = stp.tile([1, RB], F32, tag="st")
            for p in range(NPAIR):
                nc.tensor.matmul(ssum[:], ones_k_sb[:], aof[:, p, :],
                                 start=(p == 0), stop=(p == NPAIR - 1))
            st_own = ssb.tile([1, 2, RB], F32, tag="stown")
            nc.vector.tensor_copy(st_own[:, 0, :], ssum[:])
            qsum = stp.tile([1, RB], F32, tag="st", name="qsum")
            for p in range(NPAIR):
                sq = sb3.tile([128, RB], BF16, tag="sq")
                nc.vector.tensor_mul(sq[:], aof[:, p, :], aof[:, p, :])
                nc.tensor.matmul(qsum[:], ones_k_sb[:], sq[:],
                                 start=(p == 0), stop=(p == NPAIR - 1))
            nc.vector.tensor_copy(st_own[:, 1, :], qsum[:])
            ar_in = dram.tile([1, 2, RB], F32, tag="arin")
            ar_out = dram.tile([1, 2, RB], F32, tag="arout")
            nc.gpsimd.dma_start(out=ar_in[:], in_=st_own[:])
            nc.gpsimd.collective_compute(
                "AllReduce", ALU.add, replica_groups=pairs,
                ins=[ar_in.opt()], outs=[ar_out.opt()])
            st_full = ssb.tile([1, 2, RB], F32, tag="stfull")
            nc.sync.dma_start(out=st_full[:], in_=ar_out[:])

            mu = ssb.tile([1, RB], F32, tag="mu")
            musq = ssb.tile([1, RB], F32, tag="musq")
            var = ssb.tile([1, RB], F32, tag="var")
            y0 = ssb.tile([1, RB], F32, tag="y0")
            nwt = ssb.tile([1, RB], F32, tag="nwt")
            rstd = ssb.tile([1, RB], F32, tag="rstd")
            nc.vector.tensor_scalar_mul(mu[:], st_full[:, 0, :], 1.0 / HID)
            nc.vector.tensor_mul(musq[:], mu[:], mu[:])
            nc.vector.tensor_scalar(musq[:], musq[:], LN_EPS, None,
                                    ALU.subtract)
            nc.vector.scalar_tensor_tensor(
                var[:], st_full[:, 1, :], 1.0 / HID, musq[:],
                op0=ALU.mult, op1=ALU.subtract)
            nc.vector.tensor_scalar(y0.bitcast(I32)[:], var.bitcast(I32)[:],
                                    1, -1,
                                    ALU.logical_shift_right, ALU.bitwise_xor)
            nc.vector.tensor_scalar(y0.bitcast(I32)[:], y0.bitcast(I32)[:],
                                    RSQRT_MAGIC + 1, None, ALU.add)
            nc.vector.tensor_mul(nwt[:], var[:], y0[:])
            nc.vector.tensor_mul(nwt[:], nwt[:], y0[:])
            nc.vector.tensor_scalar(nwt[:], nwt[:], -0.5, 1.5,
                                    ALU.mult, ALU.add)
            nc.vector.tensor_mul(rstd[:], y0[:], nwt[:])
            mu_b = ssb.tile([1, RB], BF16, tag="mub")
            rstd_b = ssb.tile([1, RB], BF16, tag="rstdb")
            nc.vector.tensor_copy(mu_b[:], mu[:])
            nc.vector.tensor_copy(rstd_b[:], rstd[:])
            mu_s = sb3.tile([128, RB], BF16, tag="mus")
            rs_s = sb3.tile([128, RB], BF16, tag="rss")
            nc.gpsimd.partition_broadcast(mu_s[:], mu_b[:])
            nc.gpsimd.partition_broadcast(rs_s[:], rstd_b[:])

            gated = sb3.tile([128, NPAIR, RB], BF16, tag="gated", bufs=1)
            for p in range(NPAIR):
                d1 = sb3.tile([128, RB], BF16, tag="d1")
                d2 = sb3.tile([128, RB], BF16, tag="d2")
                nc.vector.tensor_sub(d1[:], aof[:, p, :], mu_s[:])
                nc.vector.tensor_mul(d2[:], d1[:], rs_s[:])
                nc.vector.tensor_mul(gated[:, p, :], d2[:],
                                     ut_sb[p][qb // 2][:, qb % 2, :])
            rs_in = dram.tile([2, CT, 128, HB], BF16, tag="rsin")
            for ctp in range(CT):
                po = opo.tile([128, RB], F32, tag=f"po{ctp % 2}")
                for p in range(NPAIR):
                    nc.tensor.matmul(
                        po[:], wout_sb[p][:, ctp * 128:(ctp + 1) * 128],
                        gated[:, p, :], start=(p == 0), stop=(p == 2))
                rt_t = sb3.tile([128, RB], BF16, tag="resid")
                nc.sync.dma_start(
                    out=rt_t[:],
                    in_=residT_r[:, ctp, qb * RB:(qb + 1) * RB])
                o_t = sb3.tile([128, 2, HB], BF16, tag="osb")
                nc.vector.tensor_add(
                    o_t.rearrange("i r j -> i (r j)")[:], po[:], rt_t[:])
                nc.gpsimd.dma_start(
                    out=rs_in[:, ctp].rearrange("r i j -> i r j"),
                    in_=o_t[:])
            rs_out = dram.tile([CT, 128, HB], BF16, tag="rsout")
            nc.gpsimd.collective_compute(
                "ReduceScatter", ALU.add, replica_groups=pairs,
                ins=[rs_in.opt()], outs=[rs_out.opt()])
            nc.sync.dma_start(out=out[qb], in_=rs_out[:])



def prep_inputs(x, attn_mask, W_proj, b_proj, ln_gamma, ln_beta, W_out, b_out):
    x = np.asarray(x, dtype=np.float32)
    W_proj = np.asarray(W_proj, dtype=np.float32)
    b_proj = np.asarray(b_proj, dtype=np.float32)
    ln_gamma = np.asarray(ln_gamma, dtype=np.float32)
    ln_beta = np.asarray(ln_beta, dtype=np.float32)
    W_out = np.asarray(W_out, dtype=np.float32)
    b_out = np.asarray(b_out, dtype=np.float32)

    tril = np.tril(np.ones((S, S), dtype=bool))
    am = np.asarray(attn_mask)
    if not all(np.array_equal(am[b], tril) for b in range(am.shape[0])):
        raise ValueError("kernel specialized for causal attn_mask")
    if np.any(b_proj != 0) or np.any(ln_beta != 0):
        raise ValueError("kernel specialized for zero b_proj / ln_beta")

    bf = ml_dtypes.bfloat16
    cos, sin = _rope_tables()
    sinN = sin.copy()
    sinN[:, 0:32] = -sinN[:, 0:32]
    cosr = np.tile(cos, (1, NH)).astype(bf)
    sinr = np.tile(sinN, (1, NH)).astype(bf)

    iw = np.arange(128)[None, :]
    ii = np.arange(128)[:, None]
    maskb = (iw >= ii).astype(bf)
    ones_k = np.ones((128, 1), dtype=bf)

    Wg = (ln_gamma[:, None] * W_out).astype(np.float32)
    U_c, V_c, Q_c, K_c = 0, HID, 2 * HID, 3 * HID

    in_maps = []
    for c in range(N_CORES):
        b, hh = c // 2, c % 2
        heads = range(NH * hh, NH * hh + NH)
        qcols = np.concatenate(
            [np.arange(Q_c + h * D, Q_c + (h + 1) * D) for h in heads])
        kcols = qcols - Q_c + K_c
        vcols = qcols - Q_c + V_c
        w_qkv = np.concatenate(
            [W_proj[:, qcols], W_proj[:, kcols], W_proj[:, vcols]],
            axis=1).astype(bf)
        d0 = hh * 384
        w_u = W_proj[:, U_c + d0:U_c + d0 + 384].astype(bf)
        w_out_own = Wg[d0:d0 + 384, :].astype(bf)
        xTb = x[b].T
        residT = ((xTb + b_out[:, None]) * 0.5).astype(bf)
        in_maps.append(dict(
            xT=np.ascontiguousarray(xTb).astype(bf),
            w_qkv=np.ascontiguousarray(w_qkv),
            w_u=np.ascontiguousarray(w_u),
            w_out=np.ascontiguousarray(w_out_own),
            cosr=cosr, sinr=sinr, maskb=maskb, ones_k=ones_k,
            residT=np.ascontiguousarray(residT),
        ))
    return in_maps


def assemble(results, B=4):
    full = np.empty((B, S, HID), dtype=np.float32)
    for c in range(N_CORES):
        b, hh = c // 2, c % 2
        o = np.asarray(results[c]["out"], dtype=np.float32)
        for qb in range(NRB):
            t0 = qb * RB + hh * HB
            full[b, t0:t0 + HB, :] = \
                o[qb].reshape(HID, HB).T
    return full


_NC_CACHE = {}


def get_nc(ndev=N_CORES):
    if ndev not in _NC_CACHE:
        pairs = [[i, i + 1] for i in range(0, ndev, 2)]
        _NC_CACHE[ndev] = build_nc(ndev, pairs)
    return _NC_CACHE[ndev]


def kernel(**inputs):
    in_maps = prep_inputs(**inputs)
    nc = get_nc(N_CORES)
    res = bass_utils.run_bass_kernel_spmd(
        nc, in_maps, core_ids=list(range(N_CORES)))
    return assemble(res.results)


# revision 24
# speedup vs baseline: 1.1115x; 1.1115x over previous
import numpy as np
import ml_dtypes

import concourse.bass as bass
import concourse.bacc as bacc
import concourse.mybir as mybir
import concourse.tile as tile
from concourse import bass_utils

BF16 = mybir.dt.bfloat16
F32 = mybir.dt.float32
I32 = mybir.dt.int32
AF = mybir.ActivationFunctionType
ALU = mybir.AluOpType

S = 2048
HID = 768
D = 64
NH = 6
NPAIR = 3
NRB = 4
RB = 512
HB = 256
CT = 6
LN_EPS = 1e-8
N_CORES = 8
RSQRT_MAGIC = 0x5F3759DF


def _rope_tables():
    inv_freq = 1.0 / (10000.0 ** (np.arange(0, D, 2, dtype=np.float64) / D))
    t = np.arange(S, dtype=np.float64)
    freqs = np.outer(t, inv_freq)
    emb = np.concatenate([freqs, freqs], axis=-1)
    return np.cos(emb).astype(np.float32), np.sin(emb).astype(np.float32)


def build_nc(ndev, pairs):
    nc = bacc.Bacc("TRN2", target_bir_lowering=False, debug=False,
                   num_devices=ndev)

    def din(name, shape, dt):
        return nc.dram_tensor(name, shape, dt, kind="ExternalInput").ap()

    xT = din("xT", [HID, S], BF16)
    w_qkv = din("w_qkv", [HID, 3 * NH * D], BF16)
    w_u = din("w_u", [HID, NPAIR * 128], BF16)
    w_out = din("w_out", [NPAIR * 128, HID], BF16)
    cosr = din("cosr", [S, NH * D], BF16)
    sinr = din("sinr", [S, NH * D], BF16)
    maskb = din("maskb", [128, 128], BF16)
    ident = din("ident", [128, 128], BF16)
    ones_k = din("ones_k", [128, 1], BF16)
    residT = din("residT", [HID, S], BF16)
    out = nc.dram_tensor("out", [NRB, CT, 128, HB], BF16,
                         kind="ExternalOutput").ap()

    xT_r = xT.rearrange("(k p) s -> k p s", p=128)
    wqkv_r = w_qkv.rearrange("(k p) c -> k p c", p=128)
    wu_r = w_u.rearrange("(k p) c -> k p c", p=128)
    wout_r = w_out.rearrange("(k p) c -> k p c", p=128)
    residT_r = residT.rearrange("(c p) s -> p c s", p=128)

    with tile.TileContext(nc) as tc:
        _emit(nc, tc, pairs, xT_r, wqkv_r, wu_r, wout_r, cosr, sinr,
              maskb, ident, ones_k, residT_r, out)
    nc.compile()
    return nc


def _emit(nc, tc, pairs, xT_r, wqkv_r, wu_r, wout_r, cosr, sinr,
          maskb, ident, ones_k, residT_r, out):
    from contextlib import ExitStack
    es = ExitStack()
    with es:
        res = es.enter_context(tc.tile_pool(name="resident", bufs=1))
        xT_sb = [res.tile([128, S], BF16, tag=f"xT{k}", name=f"xT{k}")
                 for k in range(6)]
        wqkv_sb = [res.tile([128, 3 * NH * D], BF16, tag=f"wqkv{k}",
                            name=f"wqkv{k}") for k in range(6)]
        wu_sb = [res.tile([128, NPAIR * 128], BF16, tag=f"wu{k}",
                          name=f"wu{k}") for k in range(6)]
        wout_sb = [res.tile([128, HID], BF16, tag=f"wout{k}",
                            name=f"wout{k}") for k in range(NPAIR)]
        maskb_sb = res.tile([128, 128], BF16, tag="maskb")
        ident_sb = res.tile([128, 128], BF16, tag="ident")
        ones_k_sb = res.tile([128, 1], BF16, tag="onesk")
        qt_sb = [res.tile([128, NPAIR, RB], BF16, tag=f"qt{i}", name=f"qt{i}")
                 for i in range(NRB)]
        kt_sb = [res.tile([128, NPAIR, RB], BF16, tag=f"kt{i}", name=f"kt{i}")
                 for i in range(NRB)]
        v_sb = [res.tile([128, 4, NH, D], BF16, tag=f"v{i}", name=f"v{i}")
                for i in range(NRB)]
        ao_sb = [res.tile([128, NPAIR, RB], BF16, tag=f"ao{i}", name=f"ao{i}")
                 for i in range(NRB)]
        ut_sb = [[res.tile([128, 2, RB], BF16, tag=f"ut{p}_{sp}",
                           name=f"ut{p}_{sp}")
                  for sp in range(2)] for p in range(NPAIR)]

        for k in range(6):
            nc.sync.dma_start(out=xT_sb[k][:], in_=xT_r[k])
            nc.sync.dma_start(out=wqkv_sb[k][:], in_=wqkv_r[k])
        nc.sync.dma_start(out=maskb_sb[:], in_=maskb[:])
        nc.sync.dma_start(out=ident_sb[:], in_=ident[:])
        nc.sync.dma_start(out=ones_k_sb[:], in_=ones_k[:])
        for k in range(6):
            nc.sync.dma_start(out=wu_sb[k][:], in_=wu_r[k])
        for p in range(NPAIR):
            nc.sync.dma_start(out=wout_sb[p][:], in_=wout_r[p])

        dram = es.enter_context(tc.tile_pool(name="ccdram", bufs=4,
                                             space="DRAM"))
        scp = es.enter_context(tc.tile_pool(name="p2sc", bufs=2,
                                            space="PSUM"))
        avp = es.enter_context(tc.tile_pool(name="p2av", bufs=1,
                                            space="PSUM"))
        atp = es.enter_context(tc.tile_pool(name="p2sb", bufs=16))
        sb = es.enter_context(tc.tile_pool(name="p1sb", bufs=2))
        with tc.tile_pool(name="p1psum", bufs=1, space="PSUM") as pp:
            for rt in range(16):
                pqk = pp.tile([128, 2, 512], F32, tag="pqk")
                for k in range(6):
                    lhs = xT_sb[k][:, rt * 128:(rt + 1) * 128]
                    st, sp = (k == 0), (k == 5)
                    nc.tensor.matmul(pqk[:, 0, 0:384], lhs,
                                     wqkv_sb[k][:, 0:384], start=st, stop=sp)
                    nc.tensor.matmul(pqk[:, 1, 0:384], lhs,
                                     wqkv_sb[k][:, 384:768], start=st,
                                     stop=sp)
                rb, rt4 = rt // 4, rt % 4
                r0, r1 = rt * 128, (rt + 1) * 128
                c0, c1 = rt4 * 128, (rt4 + 1) * 128
                q_t = sb.tile([128, NH, D], BF16, tag="qrow")
                k_t = sb.tile([128, NH, D], BF16, tag="krow")
                nc.scalar.copy(q_t[:], pqk[:, 0, 0:384])
                nc.scalar.copy(k_t[:], pqk[:, 1, 0:384])
                pv = pp.tile([128, 2, 512], F32, tag="pqk", name="pv")
                for k in range(6):
                    nc.tensor.matmul(pv[:, 0, 0:384],
                                     xT_sb[k][:, rt * 128:(rt + 1) * 128],
                                     wqkv_sb[k][:, 768:1152],
                                     start=(k == 0), stop=(k == 5))
                nc.vector.tensor_copy(v_sb[rb][:, rt4, :, :],
                                      pv[:, 0, 0:384])
                cos_t = sb.tile([128, NH, D], BF16, tag="cos")
                sin_t = sb.tile([128, NH, D], BF16, tag="sin")
                nc.sync.dma_start(out=cos_t[:], in_=cosr[r0:r1, :])
                nc.sync.dma_start(out=sin_t[:], in_=sinr[r0:r1, :])
                for name, src in (("q", q_t), ("k", k_t)):
                    t1 = sb.tile([128, NH, D], BF16, tag="rope_t1")
                    t2 = sb.tile([128, NH, D], BF16, tag="rope_t2")
                    rr = sb.tile([128, NH, D], BF16, tag="rope_r")
                    nc.vector.tensor_mul(t1[:, :, 0:32], src[:, :, 32:64],
                                         sin_t[:, :, 0:32])
                    nc.vector.tensor_mul(t1[:, :, 32:64], src[:, :, 0:32],
                                         sin_t[:, :, 32:64])
                    nc.vector.tensor_mul(t2[:], src[:], cos_t[:])
                    nc.vector.tensor_add(rr[:], t1[:], t2[:])
                    dst = qt_sb if name == "q" else kt_sb
                    for p in range(NPAIR):
                        tp = pp.tile([128, 128], BF16, tag="tp", name="tp")
                        nc.tensor.transpose(tp[:], rr[:, 2 * p:2 * p + 2, :],
                                            ident_sb[:])
                        nc.vector.tensor_copy(dst[rb][:, p, c0:c1], tp[:])

            for p in range(NPAIR):
                for sp in range(2):
                    pu = pp.tile([128, 2, 512], F32, tag="pqk", name="pu")
                    for k in range(6):
                        lhsw = wu_sb[k][:, p * 128:(p + 1) * 128]
                        st, spp = (k == 0), (k == 5)
                        nc.tensor.matmul(
                            pu[:, 0, :], lhsw,
                            xT_sb[k][:, 2 * sp * RB:2 * sp * RB + RB],
                            start=st, stop=spp)
                        nc.tensor.matmul(
                            pu[:, 1, :], lhsw,
                            xT_sb[k][:, (2 * sp + 1) * RB:
                                       (2 * sp + 2) * RB],
                            start=st, stop=spp)
                    usig = sb.tile([128, 2, RB], BF16, tag="usig")
                    nc.scalar.activation(usig[:], pu[:], AF.Sigmoid)
                    nc.vector.tensor_mul(ut_sb[p][sp][:], usig[:], pu[:])

        stp = es.enter_context(tc.tile_pool(name="p3st", bufs=1,
                                            space="PSUM"))
        opo = es.enter_context(tc.tile_pool(name="p3o", bufs=1,
                                            space="PSUM"))
        sb3 = es.enter_context(tc.tile_pool(name="p3sb", bufs=2))
        ssb = es.enter_context(tc.tile_pool(name="p3small", bufs=1))
        for qb in range(NRB):
            nkc = 4 * qb + 4
            for p in range(NPAIR):
                av = avp.tile([128, RB], F32, tag="av")
                ats = {}
                for kc in range(nkc):
                    t = kc - 4 * qb
                    q0 = max(t, 0) * 128
                    sc = scp.tile([128, 2, RB], F32, tag="sc")
                    at = atp.tile([128, 2, RB], BF16, tag="at")
                    kslc = kt_sb[kc // 4]
                    for h01 in range(2):
                        b0 = 64 * h01
                        nc.tensor.matmul(
                            sc[:, h01, :],
                            kslc[b0:b0 + 64, p,
                                 (kc % 4) * 128:(kc % 4 + 1) * 128],
                            qt_sb[qb][b0:b0 + 64, p, :],
                            start=True, stop=True)
                    nc.scalar.activation(at[:, :, q0:RB], sc[:, :, q0:RB],
                                         AF.Sigmoid, scale=0.125)
                    if t >= 0:
                        if q0 > 0:
                            nc.vector.memset(at[:, :, 0:q0], 0.0)
                        for h01 in range(2):
                            nc.vector.tensor_mul(
                                at[:, h01, q0:q0 + 128],
                                at[:, h01, q0:q0 + 128],
                                maskb_sb[:])
                    ats[kc] = at
                for kc in range(nkc):
                    at = ats[kc]
                    for h01 in range(2):
                        b0 = 64 * h01
                        nc.tensor.matmul(
                            av[b0:b0 + 64, :],
                            v_sb[kc // 4][:, kc % 4, 2 * p + h01, :],
                            at[:, h01, :],
                            start=(kc == 0), stop=(kc == nkc - 1),
                            skip_group_check=True)
                nc.vector.tensor_copy(ao_sb[qb][:, p, :], av[:])

            aof = ao_sb[qb]
            ssum = stp.tile([1, RB], F32, tag="st")
            for p in range(NPAIR):
                nc.tensor.matmul(ssum[:], ones_k_sb[:], aof[:, p, :],
                                 start=(p == 0), stop=(p == NPAIR - 1))
            st_own = ssb.tile([1, 2, RB], F32, tag="stown")
            nc.vector.tensor_copy(st_own[:, 0, :], ssum[:])
            qsum = stp.tile([1, RB], F32, tag="st", name="qsum")
            for p in range(NPAIR):
                sq = sb3.tile([128, RB], BF16, tag="sq")
                nc.vector.tensor_mul(sq[:], aof[:, p, :], aof[:, p, :])
                nc.tensor.matmul(qsum[:], ones_k_sb[:], sq[:],
                                 start=(p == 0), stop=(p == NPAIR - 1))
            nc.vector.tensor_copy(st_own[:, 1, :], qsum[:])
            ar_in = dram.tile([1, 2, RB], F32, tag="arin")
            ar_out = dram.tile([1, 2, RB], F32, tag="arout")
            nc.gpsimd.dma_start(out=ar_in[:], in_=st_own[:])
            nc.gpsimd.collective_compute(
                "AllReduce", ALU.add, replica_groups=pairs,
                ins=[ar_in.opt()], outs=[ar_out.opt()])
            st_full = ssb.tile([1, 2, RB], F32, tag="stfull")
            nc.sync.dma_start(out=st_full[:], in_=ar_out[:])

            mu = ssb.tile([1, RB], F32, tag="mu")
            musq = ssb.tile([1, RB], F32, tag="musq")
            var = ssb.tile([1, RB], F32, tag="var")
            y0 = ssb.tile([1, RB], F32, tag="y0")
            nwt = ssb.tile([1, RB], F32, tag="nwt")
            rstd = ssb.tile([1, RB], F32, tag="rstd")
            nc.vector.tensor_scalar_mul(mu[:], st_full[:, 0, :], 1.0 / HID)
            nc.vector.tensor_mul(musq[:], mu[:], mu[:])
            nc.vector.tensor_scalar(musq[:], musq[:], LN_EPS, None,
                                    ALU.subtract)
            nc.vector.scalar_tensor_tensor(
                var[:], st_full[:, 1, :], 1.0 / HID, musq[:],
                op0=ALU.mult, op1=ALU.subtract)
            nc.vector.tensor_scalar(y0.bitcast(I32)[:], var.bitcast(I32)[:],
                                    1, -1,
                                    ALU.logical_shift_right, ALU.bitwise_xor)
            nc.vector.tensor_scalar(y0.bitcast(I32)[:], y0.bitcast(I32)[:],
                                    RSQRT_MAGIC + 1, None, ALU.add)
            nc.vector.tensor_mul(nwt[:], var[:], y0[:])
            nc.vector.tensor_mul(nwt[:], nwt[:], y0[:])
            nc.vector.tensor_scalar(nwt[:], nwt[:], -0.5, 1.5,
                                    ALU.mult, ALU.add)
            nc.vector.tensor_mul(rstd[:], y0[:], nwt[:])
            mu_b = ssb.tile([1, RB], BF16, tag="mub")
            rstd_b = ssb.tile([1, RB], BF16, tag="rstdb")
            nc.vector.tensor_copy(mu_b[:], mu[:])
            nc.vector.tensor_copy(rstd_b[:], rstd[:])
            mu_s = sb3.tile([128, RB], BF16, tag="mus")
            rs_s = sb3.tile([128, RB], BF16, tag="rss")
            nc.gpsimd.partition_broadcast(mu_s[:], mu_b[:])
            nc.gpsimd.partition_broadcast(rs_s[:], rstd_b[:])

            gated = sb3.tile([128, NPAIR, RB], BF16, tag="gated", bufs=1)
            for p in range(NPAIR):
                d1 = sb3.tile([128, RB], BF16, tag="d1")
                d2 = sb3.tile([128, RB], BF16, tag="d2")
                nc.vector.tensor_sub(d1[:], aof[:, p, :], mu_s[:])
                nc.vector.tensor_mul(d2[:], d1[:], rs_s[:])
                nc.vector.tensor_mul(gated[:, p, :], d2[:],
                                     ut_sb[p][qb // 2][:, qb % 2, :])
            rs_in = dram.tile([2, CT, 128, HB], BF16, tag="rsin")
            for ctp in range(CT):
                po = opo.tile([128, RB], F32, tag=f"po{ctp % 2}")
                for p in range(NPAIR):
                    nc.tensor.matmul(
                        po[:], wout_sb[p][:, ctp * 128:(ctp + 1) * 128],
                        gated[:, p, :], start=(p == 0), stop=(p == 2))
                rt_t = sb3.tile([128, RB], BF16, tag="resid")
                nc.sync.dma_start(
                    out=rt_t[:],
                    in_=residT_r[:, ctp, qb * RB:(qb + 1) * RB])
                o_t = sb3.tile([128, 2, HB], BF16, tag="osb")
                nc.vector.tensor_add(
                    o_t.rearrange("i r j -> i (r j)")[:], po[:], rt_t[:])
                nc.gpsimd.dma_start(
                    out=rs_in[:, ctp].rearrange("r i j -> i r j"),
                    in_=o_t[:])
            rs_out = dram.tile([CT, 128, HB], BF16, tag="rsout")
            nc.gpsimd.collective_compute(
                "ReduceScatter", ALU.add, replica_groups=pairs,
                ins=[rs_in.opt()], outs=[rs_out.opt()])
            nc.sync.dma_start(out=out[qb], in_=rs_out[:])



def prep_inputs(x, attn_mask, W_proj, b_proj, ln_gamma, ln_beta, W_out, b_out):
    x = np.asarray(x, dtype=np.float32)
    W_proj = np.asarray(W_proj, dtype=np.float32)
    b_proj = np.asarray(b_proj, dtype=np.float32)
    ln_gamma = np.asarray(ln_gamma, dtype=np.float32)
    ln_beta = np.asarray(ln_beta, dtype=np.float32)
    W_out = np.asarray(W_out, dtype=np.float32)
    b_out = np.asarray(b_out, dtype=np.float32)

    tril = np.tril(np.ones((S, S), dtype=bool))
    am = np.asarray(attn_mask)
    if not all(np.array_equal(am[b], tril) for b in range(am.shape[0])):
        raise ValueError("kernel specialized for causal attn_mask")
    if np.any(b_proj != 0) or np.any(ln_beta != 0):
        raise ValueError("kernel specialized for zero b_proj / ln_beta")

    bf = ml_dtypes.bfloat16
    cos, sin = _rope_tables()
    sinN = sin.copy()
    sinN[:, 0:32] = -sinN[:, 0:32]
    cosr = np.tile(cos, (1, NH)).astype(bf)
    sinr = np.tile(sinN, (1, NH)).astype(bf)

    iw = np.arange(128)[None, :]
    ii = np.arange(128)[:, None]
    maskb = (iw >= ii).astype(bf)
    ident = np.eye(128, dtype=bf)
    ones_k = np.ones((128, 1), dtype=bf)

    Wg = (ln_gamma[:, None] * W_out).astype(np.float32)
    U_c, V_c, Q_c, K_c = 0, HID, 2 * HID, 3 * HID

    in_maps = []
    for c in range(N_CORES):
        b, hh = c // 2, c % 2
        heads = range(NH * hh, NH * hh + NH)
        qcols = np.concatenate(
            [np.arange(Q_c + h * D, Q_c + (h + 1) * D) for h in heads])
        kcols = qcols - Q_c + K_c
        vcols = qcols - Q_c + V_c
        w_qkv = np.concatenate(
            [W_proj[:, qcols], W_proj[:, kcols], W_proj[:, vcols]],
            axis=1).astype(bf)
        d0 = hh * 384
        w_u = W_proj[:, U_c + d0:U_c + d0 + 384].astype(bf)
        w_out_own = Wg[d0:d0 + 384, :].astype(bf)
        xTb = x[b].T
        residT = ((xTb + b_out[:, None]) * 0.5).astype(bf)
        in_maps.append(dict(
            xT=np.ascontiguousarray(xTb).astype(bf),
            w_qkv=np.ascontiguousarray(w_qkv),
            w_u=np.ascontiguousarray(w_u),
            w_out=np.ascontiguousarray(w_out_own),
            cosr=cosr, sinr=sinr, maskb=maskb, ident=ident, ones_k=ones_k,
            residT=np.ascontiguousarray(residT),
        ))
    return in_maps


def assemble(results, B=4):
    full = np.empty((B, S, HID), dtype=np.float32)
    for c in range(N_CORES):
        b, hh = c // 2, c % 2
        o = np.asarray(results[c]["out"], dtype=np.float32)
        for qb in range(NRB):
            t0 = qb * RB + hh * HB
            full[b, t0:t0 + HB, :] = \
                o[qb].reshape(HID, HB).T
    return full


_NC_CACHE = {}


def get_nc(ndev=N_CORES):
    if ndev not in _NC_CACHE:
        pairs = [[i, i + 1] for i in range(0, ndev, 2)]
        _NC_CACHE[ndev] = build_nc(ndev, pairs)
    return _NC_CACHE[ndev]


def kernel(**inputs):
    in_maps = prep_inputs(**inputs)
    nc = get_nc(N_CORES)
    res = bass_utils.run_bass_kernel_spmd(
        nc, in_maps, core_ids=list(range(N_CORES)))
    return assemble(res.results)


# revision 27
# speedup vs baseline: 1.3487x; 1.2134x over previous
import numpy as np
import ml_dtypes

import concourse.bass as bass
import concourse.bacc as bacc
import concourse.mybir as mybir
import concourse.tile as tile
from concourse import bass_utils

BF16 = mybir.dt.bfloat16
F32 = mybir.dt.float32
I32 = mybir.dt.int32
AF = mybir.ActivationFunctionType
ALU = mybir.AluOpType

S = 2048
HID = 768
D = 64
NH = 6
NPAIR = 3
NRB = 4
RB = 512
HB = 256
CT = 6
LN_EPS = 1e-8
N_CORES = 8
RSQRT_MAGIC = 0x5F3759DF


def _rope_tables():
    inv_freq = 1.0 / (10000.0 ** (np.arange(0, D, 2, dtype=np.float64) / D))
    t = np.arange(S, dtype=np.float64)
    freqs = np.outer(t, inv_freq)
    emb = np.concatenate([freqs, freqs], axis=-1)
    return np.cos(emb).astype(np.float32), np.sin(emb).astype(np.float32)


def build_nc(ndev, pairs):
    nc = bacc.Bacc("TRN2", target_bir_lowering=False, debug=False,
                   num_devices=ndev)

    def din(name, shape, dt):
        return nc.dram_tensor(name, shape, dt, kind="ExternalInput").ap()

    xT = din("xT", [HID, S], BF16)
    w_qkv = din("w_qkv", [HID, 3 * NH * D], BF16)
    w_u = din("w_u", [HID, NPAIR * 128], BF16)
    w_out = din("w_out", [NPAIR * 128, HID], BF16)
    cosr = din("cosr", [S, NH * D], BF16)
    sinr = din("sinr", [S, NH * D], BF16)
    maskb = din("maskb", [128, 128], BF16)
    ident = din("ident", [128, 128], BF16)
    ones_k = din("ones_k", [128, 1], BF16)
    residT = din("residT", [HID, S], BF16)
    out = nc.dram_tensor("out", [NRB, CT, 128, HB], BF16,
                         kind="ExternalOutput").ap()

    xT_r = xT.rearrange("(k p) s -> k p s", p=128)
    wqkv_r = w_qkv.rearrange("(k p) c -> k p c", p=128)
    wu_r = w_u.rearrange("(k p) c -> k p c", p=128)
    wout_r = w_out.rearrange("(k p) c -> k p c", p=128)
    residT_r = residT.rearrange("(c p) s -> p c s", p=128)

    with tile.TileContext(nc) as tc:
        _emit(nc, tc, pairs, xT_r, wqkv_r, wu_r, wout_r, cosr, sinr,
              maskb, ident, ones_k, residT_r, out)
    nc.compile()
    return nc


def _emit(nc, tc, pairs, xT_r, wqkv_r, wu_r, wout_r, cosr, sinr,
          maskb, ident, ones_k, residT_r, out):
    from contextlib import ExitStack
    es = ExitStack()
    with es:
        res = es.enter_context(tc.tile_pool(name="resident", bufs=1))
        xT_sb = [res.tile([128, S], BF16, tag=f"xT{k}", name=f"xT{k}")
                 for k in range(6)]
        wqkv_sb = [res.tile([128, 3 * NH * D], BF16, tag=f"wqkv{k}",
                            name=f"wqkv{k}") for k in range(6)]
        wu_sb = [res.tile([128, NPAIR * 128], BF16, tag=f"wu{k}",
                          name=f"wu{k}") for k in range(6)]
        wout_sb = [res.tile([128, HID], BF16, tag=f"wout{k}",
                            name=f"wout{k}") for k in range(NPAIR)]
        maskb_sb = res.tile([128, 128], BF16, tag="maskb")
        ident_sb = res.tile([128, 128], BF16, tag="ident")
        ones_k_sb = res.tile([128, 1], BF16, tag="onesk")
        qt_sb = [res.tile([128, NPAIR, RB], BF16, tag=f"qt{i}", name=f"qt{i}")
                 for i in range(NRB)]
        kt_sb = [res.tile([128, NPAIR, RB], BF16, tag=f"kt{i}", name=f"kt{i}")
                 for i in range(NRB)]
        v_sb = [res.tile([128, 4, NH, D], BF16, tag=f"v{i}", name=f"v{i}")
                for i in range(NRB)]
        ao_sb = [[res.tile([128, RB], BF16, tag=f"ao{i}_{p}",
                           name=f"ao{i}_{p}") for p in range(NPAIR)]
                 for i in range(NRB)]
        ut_sb = [[res.tile([128, 2, RB], BF16, tag=f"ut{p}_{sp}",
                           name=f"ut{p}_{sp}")
                  for sp in range(2)] for p in range(NPAIR)]

        for k in range(6):
            nc.sync.dma_start(out=xT_sb[k][:], in_=xT_r[k])
            nc.sync.dma_start(out=wqkv_sb[k][:], in_=wqkv_r[k])
        nc.sync.dma_start(out=maskb_sb[:], in_=maskb[:])
        nc.sync.dma_start(out=ident_sb[:], in_=ident[:])
        nc.sync.dma_start(out=ones_k_sb[:], in_=ones_k[:])
        for k in range(6):
            nc.sync.dma_start(out=wu_sb[k][:], in_=wu_r[k])
        for p in range(NPAIR):
            nc.sync.dma_start(out=wout_sb[p][:], in_=wout_r[p])

        dram = es.enter_context(tc.tile_pool(name="ccdram", bufs=4,
                                             space="DRAM"))
        scp = es.enter_context(tc.tile_pool(name="p2sc", bufs=2,
                                            space="PSUM"))
        avp = es.enter_context(tc.tile_pool(name="p2av", bufs=1,
                                            space="PSUM"))
        atp = es.enter_context(tc.tile_pool(name="p2sb", bufs=16))
        sb = es.enter_context(tc.tile_pool(name="p1sb", bufs=2))
        sb3 = es.enter_context(tc.tile_pool(name="p3sb", bufs=2))
        ssb = es.enter_context(tc.tile_pool(name="p3small", bufs=1))
        with tc.tile_pool(name="p1psum", bufs=1, space="PSUM") as pp:
            for rt in range(16):
                pqk = pp.tile([128, 2, 512], F32, tag="pqk")
                for k in range(6):
                    lhs = xT_sb[k][:, rt * 128:(rt + 1) * 128]
                    st, sp = (k == 0), (k == 5)
                    nc.tensor.matmul(pqk[:, 0, 0:384], lhs,
                                     wqkv_sb[k][:, 0:384], start=st, stop=sp)
                    nc.tensor.matmul(pqk[:, 1, 0:384], lhs,
                                     wqkv_sb[k][:, 384:768], start=st,
                                     stop=sp)
                rb, rt4 = rt // 4, rt % 4
                r0, r1 = rt * 128, (rt + 1) * 128
                c0, c1 = rt4 * 128, (rt4 + 1) * 128
                qk_t = sb.tile([128, 2, NH, D], BF16, tag="qkrow")
                nc.scalar.copy(qk_t[:], pqk[:, :, 0:384])
                q_t, k_t = qk_t[:, 0], qk_t[:, 1]
                pv = pp.tile([128, 384], F32, tag="tpv", name="pv")
                for k in range(6):
                    nc.tensor.matmul(pv[:],
                                     xT_sb[k][:, rt * 128:(rt + 1) * 128],
                                     wqkv_sb[k][:, 768:1152],
                                     start=(k == 0), stop=(k == 5))
                nc.vector.tensor_copy(v_sb[rb][:, rt4, :, :], pv[:])
                cos_t = sb.tile([128, NH, D], BF16, tag="cos")
                sin_t = sb.tile([128, NH, D], BF16, tag="sin")
                nc.sync.dma_start(out=cos_t[:], in_=cosr[r0:r1, :])
                nc.sync.dma_start(out=sin_t[:], in_=sinr[r0:r1, :])
                rrs = []
                for name, srcq in (("q", q_t), ("k", k_t)):
                    t1 = sb.tile([128, NH, D], BF16, tag="rope_t1")
                    t2 = sb.tile([128, NH, D], BF16, tag="rope_t2")
                    rr = sb.tile([128, NH, D], BF16, tag="rope_r")
                    nc.vector.tensor_mul(t1[:, :, 0:32], srcq[:, :, 32:64],
                                         sin_t[:, :, 0:32])
                    nc.vector.tensor_mul(t1[:, :, 32:64], srcq[:, :, 0:32],
                                         sin_t[:, :, 32:64])
                    nc.vector.tensor_mul(t2[:], srcq[:], cos_t[:])
                    nc.vector.tensor_add(rr[:], t1[:], t2[:])
                    rrs.append(rr)
                tpa = pp.tile([128, 2, NPAIR, 128], BF16, tag="tpv",
                              name="tpa")
                for qk in range(2):
                    for p in range(NPAIR):
                        nc.tensor.transpose(
                            tpa[:, qk, p, :],
                            rrs[qk][:, 2 * p:2 * p + 2, :], ident_sb[:])
                nc.vector.tensor_copy(qt_sb[rb][:, :, c0:c1], tpa[:, 0])
                nc.vector.tensor_copy(kt_sb[rb][:, :, c0:c1], tpa[:, 1])

        opo = es.enter_context(tc.tile_pool(name="p3o", bufs=1,
                                            space="PSUM"))
        for p in range(NPAIR):
            for sp in range(2):
                pu0 = opo.tile([128, RB], F32, tag="po0", name="pu0")
                pu1 = opo.tile([128, RB], F32, tag="po1", name="pu1")
                for k in range(6):
                    lhsw = wu_sb[k][:, p * 128:(p + 1) * 128]
                    st, spp = (k == 0), (k == 5)
                    nc.tensor.matmul(
                        pu0[:], lhsw,
                        xT_sb[k][:, 2 * sp * RB:2 * sp * RB + RB],
                        start=st, stop=spp)
                    nc.tensor.matmul(
                        pu1[:], lhsw,
                        xT_sb[k][:, (2 * sp + 1) * RB:(2 * sp + 2) * RB],
                        start=st, stop=spp)
                for j, pu in enumerate((pu0, pu1)):
                    usig = sb.tile([128, RB], BF16, tag="usig")
                    nc.scalar.activation(usig[:], pu[:], AF.Sigmoid)
                    nc.vector.tensor_mul(ut_sb[p][sp][:, j, :], usig[:],
                                         pu[:])

        for qb in range(NRB):
            nkc = 4 * qb + 4
            for p in range(NPAIR):
                av = avp.tile([128, RB], F32, tag="av")
                ats = {}
                for kc in range(nkc):
                    t = kc - 4 * qb
                    q0 = max(t, 0) * 128
                    sc = scp.tile([128, 2, RB], F32, tag="sc")
                    at = atp.tile([128, 2, RB], BF16, tag="at")
                    kslc = kt_sb[kc // 4]
                    for h01 in range(2):
                        b0 = 64 * h01
                        nc.tensor.matmul(
                            sc[:, h01, :],
                            kslc[b0:b0 + 64, p,
                                 (kc % 4) * 128:(kc % 4 + 1) * 128],
                            qt_sb[qb][b0:b0 + 64, p, :],
                            start=True, stop=True)
                    nc.scalar.activation(at[:, :, q0:RB], sc[:, :, q0:RB],
                                         AF.Sigmoid, scale=0.125)
                    if t >= 0:
                        if q0 > 0:
                            nc.gpsimd.memset(at[:, :, 0:q0], 0.0)
                        for h01 in range(2):
                            nc.vector.tensor_mul(
                                at[:, h01, q0:q0 + 128],
                                at[:, h01, q0:q0 + 128],
                                maskb_sb[:])
                    ats[kc] = at
                for kc in range(nkc):
                    at = ats[kc]
                    for h01 in range(2):
                        b0 = 64 * h01
                        nc.tensor.matmul(
                            av[b0:b0 + 64, :],
                            v_sb[kc // 4][:, kc % 4, 2 * p + h01, :],
                            at[:, h01, :],
                            start=(kc == 0), stop=(kc == nkc - 1),
                            skip_group_check=True)
                nc.vector.tensor_copy(ao_sb[qb][p][:], av[:])

            aof = ao_sb[qb]
            ssum = avp.tile([1, RB], F32, tag="av", name="ssum")
            for p in range(NPAIR):
                nc.tensor.matmul(ssum[:], ones_k_sb[:], aof[p][:],
                                 start=(p == 0), stop=(p == NPAIR - 1))
            st_own = ssb.tile([1, 2, RB], F32, tag="stown")
            nc.vector.tensor_scalar_mul(st_own[:, 0, :], ssum[:], 1.0 / HID)
            qsum = avp.tile([1, RB], F32, tag="av", name="qsum")
            for p in range(NPAIR):
                sq = sb3.tile([128, RB], BF16, tag="sq")
                nc.vector.tensor_mul(sq[:], aof[p][:], aof[p][:])
                nc.tensor.matmul(qsum[:], ones_k_sb[:], sq[:],
                                 start=(p == 0), stop=(p == NPAIR - 1))
            nc.vector.tensor_scalar_mul(st_own[:, 1, :], qsum[:], 1.0 / HID)
            ar_in = dram.tile([1, 2, RB], F32, tag="arin")
            ar_out = dram.tile([1, 2, RB], F32, tag="arout")
            nc.gpsimd.dma_start(out=ar_in[:], in_=st_own[:])
            nc.gpsimd.collective_compute(
                "AllReduce", ALU.add, replica_groups=pairs,
                ins=[ar_in.opt()], outs=[ar_out.opt()])
            st_full = ssb.tile([1, 2, RB], F32, tag="stfull")
            nc.sync.dma_start(out=st_full[:], in_=ar_out[:])

            musq = ssb.tile([1, RB], F32, tag="musq")
            var = ssb.tile([1, RB], F32, tag="var")
            y0 = ssb.tile([1, RB], F32, tag="y0")
            nwt = ssb.tile([1, RB], F32, tag="nwt")
            rstd = ssb.tile([1, RB], F32, tag="rstd")
            mu = st_full[:, 0, :]
            nc.vector.tensor_mul(musq[:], mu, mu)
            nc.vector.tensor_sub(var[:], st_full[:, 1, :], musq[:])
            nc.vector.tensor_scalar(y0.bitcast(I32)[:], var.bitcast(I32)[:],
                                    1, -1,
                                    ALU.logical_shift_right, ALU.bitwise_xor)
            nc.vector.tensor_scalar(y0.bitcast(I32)[:], y0.bitcast(I32)[:],
                                    RSQRT_MAGIC + 1, None, ALU.add)
            nc.vector.tensor_mul(nwt[:], var[:], y0[:])
            nc.vector.tensor_mul(nwt[:], nwt[:], y0[:])
            nc.vector.tensor_scalar(nwt[:], nwt[:], -0.5, 1.5,
                                    ALU.mult, ALU.add)
            nc.vector.tensor_mul(rstd[:], y0[:], nwt[:])
            mu_b = ssb.tile([1, RB], BF16, tag="mub")
            rstd_b = ssb.tile([1, RB], BF16, tag="rstdb")
            nc.vector.tensor_copy(mu_b[:], mu)
            nc.vector.tensor_copy(rstd_b[:], rstd[:])
            mu_s = sb3.tile([128, RB], BF16, tag="mus")
            rs_s = sb3.tile([128, RB], BF16, tag="rss")
            nc.gpsimd.partition_broadcast(mu_s[:], mu_b[:])
            nc.gpsimd.partition_broadcast(rs_s[:], rstd_b[:])

            gated = sb3.tile([128, NPAIR, RB], BF16, tag="gated", bufs=1)
            for p in range(NPAIR):
                d1 = sb3.tile([128, RB], BF16, tag="d1")
                d2 = sb3.tile([128, RB], BF16, tag="d2")
                nc.vector.tensor_sub(d1[:], aof[p][:], mu_s[:])
                nc.vector.tensor_mul(d2[:], d1[:], rs_s[:])
                nc.vector.tensor_mul(gated[:, p, :], d2[:],
                                     ut_sb[p][qb // 2][:, qb % 2, :])
            rs_in = dram.tile([2, CT, 128, HB], BF16, tag="rsin")
            ot_all = sb3.tile([128, CT, 2, HB], BF16, tag="osb", bufs=1)
            for ctp in range(CT):
                po = opo.tile([128, RB], F32, tag=f"po{ctp % 2}")
                for p in range(NPAIR):
                    nc.tensor.matmul(
                        po[:], wout_sb[p][:, ctp * 128:(ctp + 1) * 128],
                        gated[:, p, :], start=(p == 0), stop=(p == 2))
                rt_t = sb3.tile([128, RB], BF16, tag="resid")
                nc.sync.dma_start(
                    out=rt_t[:],
                    in_=residT_r[:, ctp, qb * RB:(qb + 1) * RB])
                nc.vector.tensor_add(
                    ot_all[:, ctp].rearrange("i r j -> i (r j)")[:],
                    po[:], rt_t[:])
            for r in range(2):
                nc.gpsimd.dma_start(
                    out=rs_in[r].rearrange("c i j -> i c j"),
                    in_=ot_all[:, :, r, :])
            rs_out = dram.tile([CT, 128, HB], BF16, tag="rsout")
            nc.gpsimd.collective_compute(
                "ReduceScatter", ALU.add, replica_groups=pairs,
                ins=[rs_in.opt()], outs=[rs_out.opt()])
            nc.sync.dma_start(out=out[qb], in_=rs_out[:])



def prep_inputs(x, attn_mask, W_proj, b_proj, ln_gamma, ln_beta, W_out, b_out):
    x = np.asarray(x, dtype=np.float32)
    W_proj = np.asarray(W_proj, dtype=np.float32)
    b_proj = np.asarray(b_proj, dtype=np.float32)
    ln_gamma = np.asarray(ln_gamma, dtype=np.float32)
    ln_beta = np.asarray(ln_beta, dtype=np.float32)
    W_out = np.asarray(W_out, dtype=np.float32)
    b_out = np.asarray(b_out, dtype=np.float32)

    tril = np.tril(np.ones((S, S), dtype=bool))
    am = np.asarray(attn_mask)
    if not all(np.array_equal(am[b], tril) for b in range(am.shape[0])):
        raise ValueError("kernel specialized for causal attn_mask")
    if np.any(b_proj != 0) or np.any(ln_beta != 0):
        raise ValueError("kernel specialized for zero b_proj / ln_beta")

    bf = ml_dtypes.bfloat16
    cos, sin = _rope_tables()
    sinN = sin.copy()
    sinN[:, 0:32] = -sinN[:, 0:32]
    cosr = np.tile(cos, (1, NH)).astype(bf)
    sinr = np.tile(sinN, (1, NH)).astype(bf)

    iw = np.arange(128)[None, :]
    ii = np.arange(128)[:, None]
    maskb = (iw >= ii).astype(bf)
    ident = np.eye(128, dtype=bf)
    ones_k = np.ones((128, 1), dtype=bf)

    Wg = (ln_gamma[:, None] * W_out).astype(np.float32)
    U_c, V_c, Q_c, K_c = 0, HID, 2 * HID, 3 * HID

    in_maps = []
    for c in range(N_CORES):
        b, hh = c // 2, c % 2
        heads = range(NH * hh, NH * hh + NH)
        qcols = np.concatenate(
            [np.arange(Q_c + h * D, Q_c + (h + 1) * D) for h in heads])
        kcols = qcols - Q_c + K_c
        vcols = qcols - Q_c + V_c
        w_qkv = np.concatenate(
            [W_proj[:, qcols], W_proj[:, kcols], W_proj[:, vcols]],
            axis=1).astype(bf)
        d0 = hh * 384
        w_u = W_proj[:, U_c + d0:U_c + d0 + 384].astype(bf)
        w_out_own = Wg[d0:d0 + 384, :].astype(bf)
        xTb = x[b].T
        residT = ((xTb + b_out[:, None]) * 0.5).astype(bf)
        in_maps.append(dict(
            xT=np.ascontiguousarray(xTb).astype(bf),
            w_qkv=np.ascontiguousarray(w_qkv),
            w_u=np.ascontiguousarray(w_u),
            w_out=np.ascontiguousarray(w_out_own),
            cosr=cosr, sinr=sinr, maskb=maskb, ident=ident, ones_k=ones_k,
            residT=np.ascontiguousarray(residT),
        ))
    return in_maps


def assemble(results, B=4):
    full = np.empty((B, S, HID), dtype=np.float32)
    for c in range(N_CORES):
        b, hh = c // 2, c % 2
        o = np.asarray(results[c]["out"], dtype=np.float32)
        for qb in range(NRB):
            t0 = qb * RB + hh * HB
            full[b, t0:t0 + HB, :] = \
                o[qb].reshape(HID, HB).T
    return full


_NC_CACHE = {}


def get_nc(ndev=N_CORES):
    if ndev not in _NC_CACHE:
        pairs = [[i, i + 1] for i in range(0, ndev, 2)]
        _NC_CACHE[ndev] = build_nc(ndev, pairs)
    return _NC_CACHE[ndev]


def kernel(**inputs):
    in_maps = prep_inputs(**inputs)
    nc = get_nc(N_CORES)
    res = bass_utils.run_bass_kernel_spmd(
        nc, in_maps, core_ids=list(range(N_CORES)))
    return assemble(res.results)
